# revision 1
# baseline (speedup 1.0000x reference)
"""Trainium2 Bass kernel for a top-2 MoE layer (B=2, T=2048, D=1024, F=4096, E=8).

Strategy (expert-parallel, per sharding hint):
  Launch 1 (router, data-parallel over tokens): each of 8 cores computes
    logits = x_slice @ Wr in fp32 on the PE, then top-2 + renormalized
    softmax combine weights on-device (DVE/ACT).  Output: combine[4096, 8].
  Host dispatch (data movement only): tokens are gathered per expert
    (all-to-all performed by the host), padded to a static capacity.
  Launch 2 (expert FFN, expert-parallel): core e holds expert e's W1/W2
    (float32r for full-rate PE matmuls), computes y = c * (gelu(x@W1+b1)@W2
    + b2) for its gathered tokens.  F is processed in 4 quarter-passes so
    weights fit in SBUF; y accumulates in SBUF across passes.
  Launch 3 (combine): out[t] = yA[t] + yB[t] — the two selected experts'
    scaled outputs per token, added on-device, data-parallel over tokens.

All arithmetic is on-device; the host only reshapes/gathers/concats.
"""

import numpy as np

import concourse.bacc as bacc
import concourse.mybir as mybir
import concourse.tile as tile
from concourse import bass_utils
from concourse.tile_rust import add_dep_helper

F32 = mybir.dt.float32
F32R = mybir.dt.float32r
AX = mybir.AxisListType
ALU = mybir.AluOpType
ACT_F = mybir.ActivationFunctionType

B, T, D, F, E = 2, 2048, 1024, 4096, 8
NTOK = B * T              # 4096
NCORES = 8
TOK_PER_CORE = NTOK // NCORES  # 512
FQ = F // 4               # F quarter = 1024

_cache = {}


def _run(nc, in_maps, trace=False, **kw):
    return bass_utils.run_bass_kernel_spmd(
        nc, in_maps, core_ids=list(range(NCORES)), trace=trace, **kw
    )


# ----------------------------------------------------------------- router ---
def build_router():
    """Per core: xT_sl [D, 512] fp32, Wr [D, E] fp32 -> comb [512, E] fp32."""
    if "router" in _cache:
        return _cache["router"]
    nc = bacc.Bacc("TRN2", target_bir_lowering=False, debug=False)
    DO = D // 128  # 8 d-slices
    TT = TOK_PER_CORE // 128  # 4 token tiles
    # packed layouts: xT_sl[p, o*512+t] = x[tok0+t, o*128+p]; Wr[p, o*8+e]
    xT_d = nc.dram_tensor("xT_sl", [128, DO * TOK_PER_CORE], F32,
                          kind="ExternalInput").ap()
    wr_d = nc.dram_tensor("Wr", [128, DO * E], F32, kind="ExternalInput").ap()
    id_d = nc.dram_tensor("id8", [E, E], F32, kind="ExternalInput").ap()
    out_d = nc.dram_tensor("comb", [128, TT * E], F32, kind="ExternalOutput").ap()

    with tile.TileContext(nc) as tc:
        with (
            tc.tile_pool(name="pool", bufs=1) as pool,
            tc.tile_pool(name="work", bufs=2) as work,
            tc.tile_pool(name="psum", bufs=2, space="PSUM") as psum,
        ):
            xT_sb = pool.tile([128, DO, TOK_PER_CORE], F32)
            wr_sb = pool.tile([128, DO, E], F32)
            id_sb = pool.tile([E, E], F32)
            comb_sb = pool.tile([128, TT, E], F32)
            nc.gpsimd.dma_start(wr_sb[:], wr_d.rearrange("p (o e) -> p o e", o=DO))
            nc.gpsimd.dma_start(id_sb[:], id_d[:])
            xt_engines = [nc.sync, nc.gpsimd, nc.sync, nc.gpsimd]
            for dh in range(4):
                off = 2 * dh * TOK_PER_CORE
                xt_engines[dh].dma_start(
                    xT_sb[:, 2 * dh:2 * dh + 2, :],
                    xT_d[:, off:off + 2 * TOK_PER_CORE].rearrange(
                        "p (o t) -> p o t", o=2),
                )

            # logits.T [E, tok] with Wr stationary (8-col weight loads), then
            # PE-transpose each 128-token tile back to [tok, E]
            lpT = psum.tile([E, TOK_PER_CORE], F32, tag="lpT")
            for do in range(DO):
                nc.tensor.matmul(
                    lpT[:],
                    wr_sb[:, do, :],
                    xT_sb[:, do, :],
                    start=(do == 0),
                    stop=(do == DO - 1),
                )
            lsbT = pool.tile([E, TOK_PER_CORE], F32)
            nc.vector.tensor_copy(lsbT[:], lpT[:])

            for tt in range(TT):
                lp = psum.tile([128, E], F32)
                nc.tensor.transpose(
                    lp[:], lsbT[:, tt * 128:(tt + 1) * 128], id_sb[:]
                )
                l = work.tile([128, E], F32, tag="l")
                nc.vector.tensor_copy(l[:], lp[:])
                mx1 = work.tile([128, 1], F32, tag="mx1")
                nc.vector.reduce_max(mx1[:], l[:], axis=AX.X)
                nmx1 = work.tile([128, 1], F32, tag="nmx1")
                nc.vector.tensor_scalar_mul(nmx1[:], mx1[:], -1.0)
                eq = work.tile([128, E], F32, tag="eq")
                nc.vector.tensor_scalar(eq[:], l[:], mx1[:], None, op0=ALU.is_equal)
                lm = work.tile([128, E], F32, tag="lm")
                nc.vector.scalar_tensor_tensor(
                    lm[:], eq[:], -1e30, l[:], op0=ALU.mult, op1=ALU.add
                )
                mx2 = work.tile([128, 1], F32, tag="mx2")
                nc.vector.reduce_max(mx2[:], lm[:], axis=AX.X)
                p = work.tile([128, E], F32, tag="p")
                nc.scalar.activation(p[:], l[:], ACT_F.Exp, bias=nmx1[:])
                e2 = work.tile([128, 1], F32, tag="e2")
                nc.scalar.activation(e2[:], mx2[:], ACT_F.Exp, bias=nmx1[:])
                den = work.tile([128, 1], F32, tag="den")
                nc.vector.tensor_scalar_add(den[:], e2[:], 1.0)
                rec = work.tile([128, 1], F32, tag="rec")
                nc.vector.reciprocal(rec[:], den[:])
                ge = work.tile([128, E], F32, tag="ge")
                nc.vector.tensor_scalar(ge[:], l[:], mx2[:], None, op0=ALU.is_ge)
                w = work.tile([128, E], F32, tag="w")
                nc.vector.tensor_scalar_mul(w[:], p[:], rec[:])
                nc.vector.tensor_mul(comb_sb[:, tt, :], w[:], ge[:])

            nc.sync.dma_start(out_d.rearrange("p (t e) -> p t e", t=TT), comb_sb[:])
    nc.compile()
    _cache["router"] = nc
    return nc


# -------------------------------------------------------------------- ffn ---
def build_ffn(cap, with_b2=True):
    """Per core (expert e): xTg [D, cap] f32r, W1 [D, F] f32r, b1 [F] f32,
    W2 [F, D] f32r, b2 [1, D] f32r, ones [1, 128] f32r, cvec [cap] f32
    -> y [cap, D] fp32 with y = cvec * (gelu(xg@W1 + b1) @ W2 + b2).

    with_b2=False specializes away the rank-1 b2 matmuls when the caller
    verified b2 is all-zero (mathematically identical output)."""
    key = ("ffn", cap, with_b2)
    if key in _cache:
        return _cache[key]
    assert cap % 32 == 0
    DO = D // 128            # 8
    TTILES = -(-cap // 128)  # token tiles (last may be partial)
    chunks = _chunk_split(cap)
    CHUNKMAX = max(cs for _, cs in chunks)

    nc = bacc.Bacc("TRN2", target_bir_lowering=False, debug=False)
    # F processed in 5 passes of eighths (1,1,2,2,2): the small first pass
    # needs only 2MB of W1 before compute can start (less HBM-inflow stall).
    EI = F // 8           # 512 f-columns per eighth
    ET = EI // 128        # 4 f-tiles per eighth
    PASSES = [(0, 2), (2, 2), (4, 2), (6, 2)]
    NPASS = len(PASSES)
    # host pre-arranges weights/activations into SBUF layout so DMAs are
    # linear per partition:
    #   xTg[p, ci-flattened (o, t)]; W1h[p, ei, o, f]; W2h[p, ei, fo, d]
    xT_d = nc.dram_tensor("xTg", [128, DO * cap], F32R, kind="ExternalInput").ap()
    w1_d = nc.dram_tensor("W1e", [128, 8, DO, EI], F32R, kind="ExternalInput").ap()
    b1_d = nc.dram_tensor("b1e", [128, F // 128], F32, kind="ExternalInput").ap()
    w2_d = nc.dram_tensor("W2e", [128, 8, ET, D], F32R, kind="ExternalInput").ap()
    b2_d = nc.dram_tensor("b2e", [1, D], F32R, kind="ExternalInput").ap()
    ones_d = nc.dram_tensor("ones", [1, 128], F32R, kind="ExternalInput").ap()
    cv_d = nc.dram_tensor("cvec", [128, TTILES], F32, kind="ExternalInput").ap()
    y_d = nc.dram_tensor("y", [TTILES * 128, D], F32, kind="ExternalOutput").ap()

    with tile.TileContext(nc) as tc:
        with (
            tc.tile_pool(name="resident", bufs=1) as res,
            tc.tile_pool(name="w1p", bufs=1) as w1p,
            tc.tile_pool(name="w2p", bufs=2) as w2p,
            tc.tile_pool(name="xtp", bufs=2) as xtp,
            tc.tile_pool(name="htp", bufs=2) as htp,
            tc.tile_pool(name="ps1", bufs=3, space="PSUM") as ps1,
            tc.tile_pool(name="ps2", bufs=2, space="PSUM") as ps2,
            tc.tile_pool(name="psw", bufs=1, space="PSUM") as psw,
        ):
            y_acc = res.tile([128, TTILES, D], F32)
            b1_sb = res.tile([128, F // 128], F32)
            b2_sb = res.tile([1, D], F32R)
            ones_sb = res.tile([1, 128], F32R)
            cv_sb = res.tile([128, TTILES], F32)
            warm_sb = res.tile([128, 512], mybir.dt.bfloat16)
            nc.scalar.dma_start(b1_sb[:], b1_d[:])
            nc.scalar.dma_start(b2_sb[:], b2_d[:])
            nc.scalar.dma_start(ones_sb[:], ones_d[:])
            nc.scalar.dma_start(cv_sb[:], cv_d[:])

            # PE warmup while the first weight DMAs land (HAM ramp)
            nc.gpsimd.memset(warm_sb[:], 0.0)
            warm_ps = psw.tile([128, 512], F32)
            for _ in range(26):
                nc.tensor.matmul(warm_ps[:], warm_sb[:, :128], warm_sb[:],
                                 start=True, stop=True)

            def load_xt(ci, c0, cs, eng=None):
                xT_sb = xtp.tile([128, DO, CHUNKMAX], F32R, tag="xt")
                off = DO * c0
                (eng or nc.sync).dma_start(
                    xT_sb[:, :, :cs],
                    xT_d[:, off:off + DO * cs].rearrange("p (o t) -> p o t", o=DO),
                )
                return xT_sb

            for pi, (ps_, cnt) in enumerate(PASSES):
                FT_p = cnt * ET
                last = pi == NPASS - 1
                w1_sb = w1p.tile([128, 2, DO, EI], F32R, tag="w1")
                w2_sb = w2p.tile([128, 2, ET, D], F32R, tag="w2")

                def load_w1(dh):
                    nc.sync.dma_start(
                        w1_sb[:, :cnt, 2 * dh:2 * dh + 2, :],
                        w1_d[:, ps_:ps_ + cnt, 2 * dh:2 * dh + 2, :],
                    )

                load_w1(0)
                xt0_sb = load_xt(0, *chunks[0]) if pi == 0 else None
                for dh in range(1, 4):
                    load_w1(dh)
                w2_dmas = [
                    nc.gpsimd.dma_start(
                        w2_sb[:, fh, :, :], w2_d[:, ps_ + fh, :, :]
                    )
                    for fh in range(cnt)
                ]
                for ci, (c0, cs) in enumerate(chunks):
                    xT_sb = xt0_sb if (pi == 0 and ci == 0) else load_xt(ci, c0, cs)
                    hT_sb = htp.tile([128, 8, CHUNKMAX], F32R, tag="ht")
                    # stage 1: hT[f, tok] = gelu(W1p.T @ xT + b1)
                    for ft in range(FT_p):
                        hp = ps1.tile([128, CHUNKMAX], F32, tag="hp")
                        for do in range(DO):
                            nc.tensor.matmul(
                                hp[:, :cs],
                                w1_sb[:, ft // ET, do,
                                      (ft % ET) * 128:(ft % ET + 1) * 128],
                                xT_sb[:, do, :cs],
                                start=(do == 0),
                                stop=(do == DO - 1),
                            )
                        g = nc.scalar.activation(
                            hT_sb[:, ft, :cs], hp[:, :cs], ACT_F.Gelu,
                            bias=b1_sb[:, ps_ * ET + ft:ps_ * ET + ft + 1],
                        )
                        if pi == 0 and ci == 0 and ft == 0:
                            # delay the W2 prefetch until the critical
                            # W1/xT transfers have finished (HBM contention)
                            for w2dma in w2_dmas:
                                add_dep_helper(
                                    g.ins, w2dma.ins,
                                    reason="pass0 W2 prefetch after stage1 start",
                                )
                    # stage 2: y[tok, d] (+)= hT.T @ W2p (+ b2 on last pass)
                    for tt in range(-(-cs // 128)):
                        gt = c0 // 128 + tt
                        m = min(128, cs - tt * 128)
                        yp = ps2.tile([128, D], F32, tag="yp")
                        for fo in range(FT_p):
                            for n in range(D // 512):
                                nc.tensor.matmul(
                                    yp[:m, n * 512:(n + 1) * 512],
                                    hT_sb[:, fo, tt * 128:tt * 128 + m],
                                    w2_sb[:, fo // ET, fo % ET,
                                          n * 512:(n + 1) * 512],
                                    start=(fo == 0),
                                    stop=(fo == FT_p - 1
                                          and not (last and with_b2)),
                                )
                        if last and with_b2:
                            for n in range(D // 512):
                                nc.tensor.matmul(
                                    yp[:m, n * 512:(n + 1) * 512],
                                    ones_sb[:, :m],
                                    b2_sb[:, n * 512:(n + 1) * 512],
                                    start=False,
                                    stop=True,
                                )
                        if pi == 0:
                            nc.vector.tensor_scalar_mul(
                                y_acc[:m, gt, :], yp[:m, :], cv_sb[:m, gt:gt + 1]
                            )
                        else:
                            nc.vector.scalar_tensor_tensor(
                                y_acc[:m, gt, :], yp[:m, :], cv_sb[:m, gt:gt + 1],
                                y_acc[:m, gt, :], op0=ALU.mult, op1=ALU.add,
                            )
                        if last:
                            nc.sync.dma_start(
                                y_d.rearrange("(t p) d -> p t d", p=128)[:m, gt, :],
                                y_acc[:m, gt, :],
                            )
    nc.compile()
    _cache[key] = nc
    return nc


# ---------------------------------------------------------------- combine ---
def build_combine():
    """Per core: packed a, b [128, (T/128)*D] fp32 -> out = a + b (same layout).

    Host packs A[t, d] -> Ah[p, tt*D + d] with t = tt*128 + p so every DMA is
    one contiguous segment per partition (cheap descriptor generation)."""
    if "comb" in _cache:
        return _cache["comb"]
    W = (TOK_PER_CORE // 128) * D  # 4096
    NP = 8  # pieces
    PW = W // NP
    nc = bacc.Bacc("TRN2", target_bir_lowering=False, debug=False)
    a_d = nc.dram_tensor("a", [128, W], F32, kind="ExternalInput").ap()
    b_d = nc.dram_tensor("b", [128, W], F32, kind="ExternalInput").ap()
    o_d = nc.dram_tensor("o", [128, W], F32, kind="ExternalOutput").ap()
    with tile.TileContext(nc) as tc:
        with tc.tile_pool(name="pool", bufs=4) as pool:
            for pc in range(NP):
                sl = slice(pc * PW, (pc + 1) * PW)
                at = pool.tile([128, PW], F32, tag="a")
                bt = pool.tile([128, PW], F32, tag="b")
                nc.sync.dma_start(at[:], a_d[:, sl])
                nc.gpsimd.dma_start(bt[:], b_d[:, sl])
                nc.vector.tensor_add(at[:], at[:], bt[:])
                nc.scalar.dma_start(o_d[:, sl], at[:])
    nc.compile()
    _cache["comb"] = nc
    return nc


# ----------------------------------------------------------------- driver ---
def _chunk_split(cap):
    """Split cap (multiple of 32) into chunks: all 128-aligned starts, sizes
    multiples of 128 except the last (multiple of 32), each >=256 and <=512."""
    full = cap // 128
    rem = cap % 128
    k = -(-cap // 512)
    counts = [full // k + (1 if i < full % k else 0) for i in range(k)]
    chunks, c0 = [], 0
    for i, n in enumerate(counts):
        cs = n * 128 + (rem if i == k - 1 else 0)
        chunks.append((c0, cs))
        c0 += cs
    return chunks


def _moe_forward(x2d, Wr, W1, b1, W2, b2, trace=False):
    """x2d: [NTOK, D] fp32. Returns (out [NTOK, D] fp32, exec_ns_total|None)."""
    DO = D // 128

    # --- launch 1: router ---
    rnc = build_router()
    wrh = np.ascontiguousarray(Wr.reshape(DO, 128, E).transpose(1, 0, 2).reshape(128, -1))
    id8 = np.eye(E, dtype=np.float32)
    in_maps = [
        {"xT_sl": np.ascontiguousarray(
            x2d[c * TOK_PER_CORE:(c + 1) * TOK_PER_CORE]
            .reshape(TOK_PER_CORE, DO, 128).transpose(2, 1, 0).reshape(128, -1)),
         "Wr": wrh, "id8": id8}
        for c in range(NCORES)
    ]
    rres = _run(rnc, in_maps, trace=trace)
    comb = np.concatenate(
        [rres.results[c]["comb"].reshape(128, TOK_PER_CORE // 128, E)
         .transpose(1, 0, 2).reshape(TOK_PER_CORE, E) for c in range(NCORES)],
        axis=0)
    exec_ns = rres.exec_time_ns or 0
    per_launch = [rres.exec_time_ns]

    # --- host dispatch (data movement only) ---
    top2 = np.argpartition(-comb, 1, axis=1)[:, :2]  # [NTOK, 2]
    sel_lists, cvals = [], []
    for e in range(E):
        sel = np.nonzero((top2 == e).any(axis=1))[0]
        sel_lists.append(sel)
        cvals.append(comb[sel, e])
    counts = np.array([len(s) for s in sel_lists])
    MAXCAP = 1664  # SBUF limit for y accumulator residency
    nbatch = max(1, -(-int(counts.max()) // MAXCAP))
    cap = int(max(256, -(-(-(-counts.max() // nbatch)) // 32) * 32))

    fnc = build_ffn(cap, with_b2=bool(np.any(b2)))
    chunks = _chunk_split(cap)
    ones_in = np.ones((1, 128), np.float32)
    w_packed = [
        {"W1e": np.ascontiguousarray(
            W1[e].reshape(DO, 128, 8, F // 8).transpose(1, 2, 0, 3)),
         "b1e": np.ascontiguousarray(b1[e].reshape(F // 128, 128).T),
         "W2e": np.ascontiguousarray(
            W2[e].reshape(8, F // (8 * 128), 128, D).transpose(2, 0, 1, 3)),
         "b2e": np.ascontiguousarray(b2[e]).reshape(1, D)}
        for e in range(E)
    ]
    ys = [np.zeros((0, D), np.float32) for _ in range(E)]
    for bi in range(nbatch):
        in_maps = []
        for e in range(E):
            sel_b = sel_lists[e][bi * cap:(bi + 1) * cap]
            cv_b = cvals[e][bi * cap:(bi + 1) * cap]
            n_e = len(sel_b)
            xsel = np.zeros((cap, D), np.float32)
            xsel[:n_e] = x2d[sel_b]
            xg = np.concatenate(
                [xsel[c0:c0 + cs].reshape(cs, DO, 128).transpose(2, 1, 0)
                 .reshape(128, -1) for (c0, cs) in chunks], axis=1)
            ttiles = -(-cap // 128)
            cv = np.zeros(ttiles * 128, np.float32)
            cv[:n_e] = cv_b
            cv = np.ascontiguousarray(cv.reshape(ttiles, 128).T)
            in_maps.append({"xTg": np.ascontiguousarray(xg), "ones": ones_in,
                            "cvec": cv, **w_packed[e]})
        fres = _run(fnc, in_maps, trace=trace)
        ys = [np.concatenate([ys[e], fres.results[e]["y"][:cap]]) for e in range(E)]
        exec_ns += fres.exec_time_ns or 0
        per_launch.append(fres.exec_time_ns)

    # --- host: build per-token (A, B) contribution rows (gather only) ---
    slot = np.zeros((NTOK, E), np.int64)
    for e in range(E):
        slot[sel_lists[e], e] = np.arange(counts[e])
    e1, e2v = top2[:, 0], top2[:, 1]
    A = np.empty((NTOK, D), np.float32)
    Bm = np.empty((NTOK, D), np.float32)
    for e in range(E):
        m1 = e1 == e
        A[m1] = ys[e][slot[m1, e]]
        m2 = e2v == e
        Bm[m2] = ys[e][slot[m2, e]]

    # --- launch 3: combine ---
    cnc = build_combine()

    def pack(m, c):
        sl = m[c * TOK_PER_CORE:(c + 1) * TOK_PER_CORE]
        return np.ascontiguousarray(
            sl.reshape(TOK_PER_CORE // 128, 128, D).transpose(1, 0, 2)
            .reshape(128, -1))

    in_maps = [{"a": pack(A, c), "b": pack(Bm, c)} for c in range(NCORES)]
    cres = _run(cnc, in_maps, trace=trace)
    out = np.concatenate(
        [cres.results[c]["o"].reshape(128, TOK_PER_CORE // 128, D)
         .transpose(1, 0, 2).reshape(TOK_PER_CORE, D) for c in range(NCORES)],
        axis=0)
    exec_ns += cres.exec_time_ns or 0
    per_launch.append(cres.exec_time_ns)
    if trace:
        print(f"per-launch exec ns (router, ffn, combine): {per_launch}")
        _moe_forward.last = (rres, fres, cres)
    return out, (exec_ns if trace else None)


def kernel(x, Wr, W1, b1, W2, b2):
    x = np.asarray(x, np.float32)
    out, _ = _moe_forward(
        x.reshape(NTOK, D),
        np.asarray(Wr, np.float32),
        np.asarray(W1, np.float32),
        np.asarray(b1, np.float32),
        np.asarray(W2, np.float32),
        np.asarray(b2, np.float32),
        trace=False,
    )
    return out.reshape(B, T, D)



# revision 2
# speedup vs baseline: 1.1760x; 1.1760x over previous
"""Trainium2 Bass kernel for a top-2 MoE layer (B=2, T=2048, D=1024, F=4096, E=8).

Strategy (expert-parallel, per sharding hint):
  Launch 1 (router, data-parallel over tokens): each of 8 cores computes
    logits = x_slice @ Wr in fp32 on the PE, then top-2 + renormalized
    softmax combine weights on-device (DVE/ACT).  Output: combine[4096, 8].
  Host dispatch (data movement only): tokens are gathered per expert
    (all-to-all performed by the host), padded to a static capacity.
  Launch 2 (expert FFN, expert-parallel): core e holds expert e's W1/W2 in
    bf16, computes yT[d, tok] = (gelu(W1.T x + b1).T W2 + b2) * cvec for its
    gathered tokens.  Both matmul stages stream TOKENS as the moving
    operand, so PE work scales with the real capacity (no pad-tile passes).
    F is processed in 4 quarter-passes so weights stream through SBUF.
  Launch 3 (combine): out[t] = yA[t] + yB[t] in bf16 — the two selected
    experts' scaled outputs per token, added on-device.

All arithmetic is on-device; the host only reshapes/gathers/casts.
"""

import numpy as np
import ml_dtypes

import concourse.bacc as bacc
import concourse.mybir as mybir
import concourse.tile as tile
from concourse import bass_utils

F32 = mybir.dt.float32
F32R = mybir.dt.float32r
BF16 = mybir.dt.bfloat16
AX = mybir.AxisListType
ALU = mybir.AluOpType
ACT_F = mybir.ActivationFunctionType

B, T, D, F, E = 2, 2048, 1024, 4096, 8
NTOK = B * T              # 4096
NCORES = 8
TOK_PER_CORE = NTOK // NCORES  # 512
NPASS = 4                 # F streamed in 4 slices of 1024 cols
FTP = (F // 128) // NPASS  # 8 f-tiles per pass
DO = D // 128             # 8
DT = D // 128             # 8 output d-tiles
NPBF = ml_dtypes.bfloat16

_cache = {}


def _run(nc, in_maps, trace=False, **kw):
    return bass_utils.run_bass_kernel_spmd(
        nc, in_maps, core_ids=list(range(NCORES)), trace=trace, **kw
    )


# ----------------------------------------------------------------- router ---
def build_router():
    """Per core: xT_sl [D, 512] fp32, Wr [D, E] fp32 -> comb [512, E] fp32."""
    if "router" in _cache:
        return _cache["router"]
    nc = bacc.Bacc("TRN2", target_bir_lowering=False, debug=False)
    DOr = D // 128  # 8 d-slices
    TT = TOK_PER_CORE // 128  # 4 token tiles
    # packed layouts: xT_sl[p, o*512+t] = x[tok0+t, o*128+p]; Wr[p, o*8+e]
    xT_d = nc.dram_tensor("xT_sl", [128, DOr * TOK_PER_CORE], F32,
                          kind="ExternalInput").ap()
    wr_d = nc.dram_tensor("Wr", [128, DOr * E], F32, kind="ExternalInput").ap()
    id_d = nc.dram_tensor("id8", [E, E], F32, kind="ExternalInput").ap()
    out_d = nc.dram_tensor("comb", [128, TT * E], F32, kind="ExternalOutput").ap()

    with tile.TileContext(nc) as tc:
        with (
            tc.tile_pool(name="pool", bufs=1) as pool,
            tc.tile_pool(name="work", bufs=4) as work,
            tc.tile_pool(name="psum", bufs=2, space="PSUM") as psum,
        ):
            xT_sb = pool.tile([128, DOr, TOK_PER_CORE], F32)
            wr_sb = pool.tile([128, DOr, E], F32)
            id_sb = pool.tile([E, E], F32)
            comb_sb = pool.tile([128, TT, E], F32)
            nc.gpsimd.dma_start(wr_sb[:], wr_d.rearrange("p (o e) -> p o e", o=DOr))
            nc.gpsimd.dma_start(id_sb[:], id_d[:])
            xt_engines = [nc.sync, nc.gpsimd, nc.sync, nc.gpsimd]
            for dh in range(4):
                off = 2 * dh * TOK_PER_CORE
                xt_engines[dh].dma_start(
                    xT_sb[:, 2 * dh:2 * dh + 2, :],
                    xT_d[:, off:off + 2 * TOK_PER_CORE].rearrange(
                        "p (o t) -> p o t", o=2),
                )

            # logits.T [E, tok] with Wr stationary (8-col weight loads), then
            # PE-transpose each 128-token tile back to [tok, E]
            lpT = psum.tile([E, TOK_PER_CORE], F32, tag="lpT")
            for do in range(DOr):
                nc.tensor.matmul(
                    lpT[:],
                    wr_sb[:, do, :],
                    xT_sb[:, do, :],
                    start=(do == 0),
                    stop=(do == DOr - 1),
                )
            lsbT = pool.tile([E, TOK_PER_CORE], F32)
            nc.vector.tensor_copy(lsbT[:], lpT[:])

            for tt in range(TT):
                lp = psum.tile([128, E], F32)
                nc.tensor.transpose(
                    lp[:], lsbT[:, tt * 128:(tt + 1) * 128], id_sb[:]
                )
                l = work.tile([128, E], F32, tag="l")
                nc.vector.tensor_copy(l[:], lp[:])
                mx1 = work.tile([128, 1], F32, tag="mx1")
                nc.vector.reduce_max(mx1[:], l[:], axis=AX.X)
                nmx1 = work.tile([128, 1], F32, tag="nmx1")
                nc.vector.tensor_scalar_mul(nmx1[:], mx1[:], -1.0)
                eq = work.tile([128, E], F32, tag="eq")
                nc.vector.tensor_scalar(eq[:], l[:], mx1[:], None, op0=ALU.is_equal)
                lm = work.tile([128, E], F32, tag="lm")
                nc.vector.scalar_tensor_tensor(
                    lm[:], eq[:], -1e30, l[:], op0=ALU.mult, op1=ALU.add
                )
                mx2 = work.tile([128, 1], F32, tag="mx2")
                nc.vector.reduce_max(mx2[:], lm[:], axis=AX.X)
                p = work.tile([128, E], F32, tag="p")
                nc.scalar.activation(p[:], l[:], ACT_F.Exp, bias=nmx1[:])
                e2 = work.tile([128, 1], F32, tag="e2")
                nc.scalar.activation(e2[:], mx2[:], ACT_F.Exp, bias=nmx1[:])
                den = work.tile([128, 1], F32, tag="den")
                nc.vector.tensor_scalar_add(den[:], e2[:], 1.0)
                rec = work.tile([128, 1], F32, tag="rec")
                nc.vector.reciprocal(rec[:], den[:])
                ge = work.tile([128, E], F32, tag="ge")
                nc.vector.tensor_scalar(ge[:], l[:], mx2[:], None, op0=ALU.is_ge)
                w = work.tile([128, E], F32, tag="w")
                nc.vector.tensor_scalar_mul(w[:], p[:], rec[:])
                nc.vector.tensor_mul(comb_sb[:, tt, :], w[:], ge[:])

            nc.sync.dma_start(out_d.rearrange("p (t e) -> p t e", t=TT), comb_sb[:])
    nc.compile()
    _cache["router"] = nc
    return nc


# -------------------------------------------------------------------- ffn ---
def build_ffn2(cap):
    """Per core (expert e), all matmul operands bf16 (fp32 PSUM accumulate):
      xTg  [128, DO*cap]  bf16   xTg[p, o*cap+t] = x_gathered[t, o*128+p]
      W1e  [128, 32, DO, 128] bf16  W1e[p, g, o, j] = W1[o*128+p, g*128+j]
      W2e  [128, 32, DT, 128] bf16  W2e[p, g, k, j] = W2[g*128+p, k*128+j]
      b1e  [128, 32] fp32   b1e[p, g] = b1[g*128+p]
      b2e  [128, DT] fp32   b2e[p, k] = b2[k*128+p]
      cvr  [128, cap] fp32  combine weight per token, replicated over partitions
    -> y [128, DT*cap] bf16, y[p, k*cap+t] = (ffn(x)[t, k*128+p]+b2)*cv[t]

    Both stages stream tokens as the moving operand so PE rows scale with
    cap (not padded 128-tiles): stage1 hT[f,tok] = gelu(W1.T @ xT + b1);
    stage2 yT[d,tok] = W2tile.T @ hT accumulated over f in PSUM, passes
    accumulated in SBUF fp32, then (y+b2)*cv fused on DVE, bf16 out."""
    key = ("ffn2", cap)
    if key in _cache:
        return _cache[key]
    assert cap % 32 == 0
    chunks = _chunk_split(cap)

    nc = bacc.Bacc("TRN2", target_bir_lowering=False, debug=False)
    xT_d = nc.dram_tensor("xTg", [128, DO * cap], BF16, kind="ExternalInput").ap()
    w1_d = nc.dram_tensor("W1e", [128, NPASS * FTP, DO, 128], BF16,
                          kind="ExternalInput").ap()
    w2_d = nc.dram_tensor("W2e", [128, NPASS * FTP, DT, 128], BF16,
                          kind="ExternalInput").ap()
    b1_d = nc.dram_tensor("b1e", [128, F // 128], F32, kind="ExternalInput").ap()
    b2_d = nc.dram_tensor("b2e", [128, DT], F32, kind="ExternalInput").ap()
    cv_d = nc.dram_tensor("cvr", [128, cap], F32, kind="ExternalInput").ap()
    y_d = nc.dram_tensor("y", [128, DT * cap], BF16, kind="ExternalOutput").ap()

    with tile.TileContext(nc) as tc:
        with (
            tc.tile_pool(name="res", bufs=1) as res,
            tc.tile_pool(name="w1p", bufs=2) as w1p,
            tc.tile_pool(name="w2p", bufs=2) as w2p,
            tc.tile_pool(name="htp", bufs=2) as htp,
            tc.tile_pool(name="ps1", bufs=3, space="PSUM") as ps1,
            tc.tile_pool(name="ps2", bufs=3, space="PSUM") as ps2,
            tc.tile_pool(name="psw", bufs=1, space="PSUM") as psw,
        ):
            xT_sb = res.tile([128, DO, cap], BF16)
            y_acc = res.tile([128, DT, cap], F32)
            y_out = res.tile([128, DT, cap], BF16)
            b1_sb = res.tile([128, F // 128], F32)
            b2_sb = res.tile([128, DT], F32)
            cv_sb = res.tile([128, cap], F32)
            warm_sb = res.tile([128, 512], BF16)

            # PE warmup while the first weight/act DMAs land (HAM ramp);
            # ~12 cold matmuls span ~5us which covers the k=4 window.
            nc.gpsimd.memset(warm_sb[:], 0.0)
            warm_ps = psw.tile([128, 512], F32)
            for _ in range(12):
                nc.tensor.matmul(warm_ps[:], warm_sb[:, :128], warm_sb[:],
                                 start=True, stop=True)

            nc.scalar.dma_start(b1_sb[:], b1_d[:])
            nc.scalar.dma_start(b2_sb[:], b2_d[:])
            nc.scalar.dma_start(cv_sb[:], cv_d[:])
            # x chunks on gpsimd (SWDGE); w2 passes queue behind them so the
            # critical stage-1 inputs take priority on this engine.
            xr = xT_d.rearrange("p (o t) -> p o t", o=DO)
            for (c0, cs) in chunks:
                nc.gpsimd.dma_start(xT_sb[:, :, c0:c0 + cs], xr[:, :, c0:c0 + cs])

            yr = y_d.rearrange("p (k t) -> p k t", k=DT)
            for p in range(NPASS):
                # W1 in per-ftile slices (256KB) so stage1 can start after the
                # first slice; alternate the two HWDGE rings.
                w1_sb = w1p.tile([128, FTP, DO, 128], BF16, tag="w1")
                for ft in range(FTP):
                    eng = nc.sync if ft % 2 == 0 else nc.scalar
                    g = p * FTP + ft
                    eng.dma_start(w1_sb[:, ft:ft + 1, :, :],
                                  w1_d[:, g:g + 1, :, :])
                w2_sb = w2p.tile([128, FTP, DT, 128], BF16, tag="w2")
                nc.gpsimd.dma_start(w2_sb[:],
                                    w2_d[:, p * FTP:(p + 1) * FTP, :, :])

                hT_sb = htp.tile([128, FTP, cap], BF16, tag="ht")
                # stage 1: hT[f, tok] = gelu(W1p.T @ xT + b1)
                for (c0, cs) in chunks:
                    for ft in range(FTP):
                        hp = ps1.tile([128, 512], F32, tag="hp")
                        for do in range(DO):
                            nc.tensor.matmul(
                                hp[:, :cs],
                                w1_sb[:, ft, do, :],
                                xT_sb[:, do, c0:c0 + cs],
                                start=(do == 0),
                                stop=(do == DO - 1),
                            )
                        g = p * FTP + ft
                        nc.scalar.activation(
                            hT_sb[:, ft, c0:c0 + cs], hp[:, :cs], ACT_F.Gelu,
                            bias=b1_sb[:, g:g + 1],
                        )
                # stage 2: yT[d, tok] += W2tile.T @ hT (tokens moving)
                for (c0, cs) in chunks:
                    for dt in range(DT):
                        yp = ps2.tile([128, 512], F32, tag="yp")
                        for ft in range(FTP):
                            nc.tensor.matmul(
                                yp[:, :cs],
                                w2_sb[:, ft, dt, :],
                                hT_sb[:, ft, c0:c0 + cs],
                                start=(ft == 0),
                                stop=(ft == FTP - 1),
                            )
                        if p == 0:
                            nc.vector.tensor_copy(
                                y_acc[:, dt, c0:c0 + cs], yp[:, :cs])
                        else:
                            nc.vector.tensor_add(
                                y_acc[:, dt, c0:c0 + cs],
                                y_acc[:, dt, c0:c0 + cs], yp[:, :cs])
                        if p == NPASS - 1:
                            nc.vector.scalar_tensor_tensor(
                                y_out[:, dt, c0:c0 + cs],
                                y_acc[:, dt, c0:c0 + cs],
                                b2_sb[:, dt:dt + 1],
                                cv_sb[:, c0:c0 + cs],
                                op0=ALU.add, op1=ALU.mult,
                            )
                            eng = nc.sync if dt % 2 == 0 else nc.scalar
                            eng.dma_start(yr[:, dt:dt + 1, c0:c0 + cs],
                                          y_out[:, dt:dt + 1, c0:c0 + cs])
    nc.compile()
    _cache[key] = nc
    return nc


# ---------------------------------------------------------------- combine ---
def build_combine2():
    """Per core: packed a, b [128, (T/128)*D] bf16 -> o = a + b (bf16).

    Host packs A[t, d] -> Ah[p, tt*D + d] with t = tt*128 + p so every DMA is
    one contiguous segment per partition."""
    if "comb2" in _cache:
        return _cache["comb2"]
    W = (TOK_PER_CORE // 128) * D  # 4096
    NP = 4  # pieces
    PW = W // NP
    nc = bacc.Bacc("TRN2", target_bir_lowering=False, debug=False)
    a_d = nc.dram_tensor("a", [128, W], BF16, kind="ExternalInput").ap()
    b_d = nc.dram_tensor("b", [128, W], BF16, kind="ExternalInput").ap()
    o_d = nc.dram_tensor("o", [128, W], BF16, kind="ExternalOutput").ap()
    with tile.TileContext(nc) as tc:
        with tc.tile_pool(name="pool", bufs=4) as pool:
            for pc in range(NP):
                sl = slice(pc * PW, (pc + 1) * PW)
                at = pool.tile([128, PW], BF16, tag="a")
                bt = pool.tile([128, PW], BF16, tag="b")
                nc.sync.dma_start(at[:], a_d[:, sl])
                nc.gpsimd.dma_start(bt[:], b_d[:, sl])
                nc.vector.tensor_add(at[:], at[:], bt[:])
                nc.scalar.dma_start(o_d[:, sl], at[:])
    nc.compile()
    _cache["comb2"] = nc
    return nc


# ----------------------------------------------------------------- driver ---
def _chunk_split(cap):
    """Split cap (multiple of 32) into chunks: all 128-aligned starts, sizes
    multiples of 128 except the last (multiple of 32), each <=512."""
    full = cap // 128
    rem = cap % 128
    k = -(-cap // 512)
    counts = [full // k + (1 if i < full % k else 0) for i in range(k)]
    chunks, c0 = [], 0
    for i, n in enumerate(counts):
        cs = n * 128 + (rem if i == k - 1 else 0)
        chunks.append((c0, cs))
        c0 += cs
    return chunks


def _pack_weights(W1, b1, W2, b2):
    """Host-side repack of expert weights into the FFN kernel layouts."""
    packed = []
    for e in range(E):
        # W1[e]: [D, F] -> [128 dpart, 32 ftile, 8 do, 128 fcol]
        w1 = np.ascontiguousarray(
            W1[e].reshape(DO, 128, NPASS * FTP, 128).transpose(1, 2, 0, 3)
            .reshape(128, NPASS * FTP, DO, 128)).astype(NPBF)
        # W2[e]: [F, D] -> [128 fpart, 32 ftile, 8 dt, 128 dcol]
        w2 = np.ascontiguousarray(
            W2[e].reshape(NPASS * FTP, 128, DT, 128).transpose(1, 0, 2, 3)
            .reshape(128, NPASS * FTP, DT, 128)).astype(NPBF)
        b1p = np.ascontiguousarray(b1[e].reshape(F // 128, 128).T).astype(np.float32)
        b2p = np.ascontiguousarray(b2[e].reshape(DT, 128).T).astype(np.float32)
        packed.append({"W1e": w1, "W2e": w2, "b1e": b1p, "b2e": b2p})
    return packed


def _moe_forward(x2d, Wr, W1, b1, W2, b2, trace=False):
    """x2d: [NTOK, D] fp32. Returns (out [NTOK, D] fp32, exec_ns_total|None)."""
    DOr = D // 128

    # --- launch 1: router ---
    rnc = build_router()
    wrh = np.ascontiguousarray(Wr.reshape(DOr, 128, E).transpose(1, 0, 2).reshape(128, -1))
    id8 = np.eye(E, dtype=np.float32)
    in_maps = [
        {"xT_sl": np.ascontiguousarray(
            x2d[c * TOK_PER_CORE:(c + 1) * TOK_PER_CORE]
            .reshape(TOK_PER_CORE, DOr, 128).transpose(2, 1, 0).reshape(128, -1)),
         "Wr": wrh, "id8": id8}
        for c in range(NCORES)
    ]
    rres = _run(rnc, in_maps, trace=trace)
    comb = np.concatenate(
        [rres.results[c]["comb"].reshape(128, TOK_PER_CORE // 128, E)
         .transpose(1, 0, 2).reshape(TOK_PER_CORE, E) for c in range(NCORES)],
        axis=0)
    exec_ns = rres.exec_time_ns or 0
    per_launch = [rres.exec_time_ns]

    # --- host dispatch (data movement only) ---
    top2 = np.argpartition(-comb, 1, axis=1)[:, :2]  # [NTOK, 2]
    sel_lists, cvals = [], []
    for e in range(E):
        sel = np.nonzero((top2 == e).any(axis=1))[0]
        sel_lists.append(sel)
        cvals.append(comb[sel, e])
    counts = np.array([len(s) for s in sel_lists])
    MAXCAP = 1280  # SBUF limit for resident activations
    nbatch = max(1, -(-int(counts.max()) // MAXCAP))
    cap = int(max(256, -(-(-(-counts.max() // nbatch)) // 32) * 32))

    fnc = build_ffn2(cap)
    w_packed = _pack_weights(W1, b1, W2, b2)
    x_bf = x2d.astype(NPBF)
    ys = [np.zeros((0, D), NPBF) for _ in range(E)]
    for bi in range(nbatch):
        in_maps = []
        for e in range(E):
            sel_b = sel_lists[e][bi * cap:(bi + 1) * cap]
            cv_b = cvals[e][bi * cap:(bi + 1) * cap]
            n_e = len(sel_b)
            xsel = np.zeros((cap, D), NPBF)
            xsel[:n_e] = x_bf[sel_b]
            xg = np.ascontiguousarray(
                xsel.reshape(cap, DO, 128).transpose(2, 1, 0).reshape(128, -1))
            cv = np.zeros(cap, np.float32)
            cv[:n_e] = cv_b
            cvr = np.ascontiguousarray(
                np.broadcast_to(cv[None, :], (128, cap)))
            in_maps.append({"xTg": xg, "cvr": cvr, **w_packed[e]})
        fres = _run(fnc, in_maps, trace=trace)
        ys = [np.concatenate([
            ys[e],
            fres.results[e]["y"].reshape(128, DT, cap).transpose(2, 1, 0)
            .reshape(cap, D)]) for e in range(E)]
        exec_ns += fres.exec_time_ns or 0
        per_launch.append(fres.exec_time_ns)

    # --- host: build per-token (A, B) contribution rows (gather only) ---
    slot = np.zeros((NTOK, E), np.int64)
    for e in range(E):
        slot[sel_lists[e], e] = np.arange(counts[e])
    e1, e2v = top2[:, 0], top2[:, 1]
    A = np.empty((NTOK, D), NPBF)
    Bm = np.empty((NTOK, D), NPBF)
    for e in range(E):
        m1 = e1 == e
        A[m1] = ys[e][slot[m1, e]]
        m2 = e2v == e
        Bm[m2] = ys[e][slot[m2, e]]

    # --- launch 3: combine ---
    cnc = build_combine2()

    def pack(m, c):
        sl = m[c * TOK_PER_CORE:(c + 1) * TOK_PER_CORE]
        return np.ascontiguousarray(
            sl.reshape(TOK_PER_CORE // 128, 128, D).transpose(1, 0, 2)
            .reshape(128, -1))

    in_maps = [{"a": pack(A, c), "b": pack(Bm, c)} for c in range(NCORES)]
    cres = _run(cnc, in_maps, trace=trace)
    out = np.concatenate(
        [cres.results[c]["o"].astype(np.float32)
         .reshape(128, TOK_PER_CORE // 128, D)
         .transpose(1, 0, 2).reshape(TOK_PER_CORE, D) for c in range(NCORES)],
        axis=0)
    exec_ns += cres.exec_time_ns or 0
    per_launch.append(cres.exec_time_ns)
    if trace:
        print(f"per-launch exec ns (router, ffn, combine): {per_launch}")
        _moe_forward.last = (rres, fres, cres)
    return out, (exec_ns if trace else None)


def kernel(x, Wr, W1, b1, W2, b2):
    x = np.asarray(x, np.float32)
    out, _ = _moe_forward(
        x.reshape(NTOK, D),
        np.asarray(Wr, np.float32),
        np.asarray(W1, np.float32),
        np.asarray(b1, np.float32),
        np.asarray(W2, np.float32),
        np.asarray(b2, np.float32),
        trace=False,
    )
    return out.reshape(B, T, D)
